# revision 18
# baseline (speedup 1.0000x reference)
"""Causal self-attention (B=4, T=2048, C=1024, H=16, D=64) on 8 TRN2 NeuronCores.

Sharding: 8 cores = 4 batches x 2 head-groups (8 heads each). Each core:
  - QKV projection for its (batch, head-group) column slice of w_attn,
    producing qT/kT in head-pair-packed [d, t] layout (head 2i in partitions
    0-63, head 2i+1 in 64-127 of pair tile i) and v in [t, d].
  - Causal attention in scoresT layout (scores^T[k, q] straight off the PE;
    K=64 matmuls via PE quadrant base-partition addressing; causal mask
    applied by accumulating -1000*triu into the scores PSUM with one extra
    128-col matmul so exp() emits zeros; softmax denominators via an
    appended ones-column on V).
  - Row-sharded output projection -> per-core partial [T, C].
Host sums the two partials per batch and adds b_proj.

All matmul operands are bf16, accumulation in fp32 PSUM. Scheduling: the PE
p-state drops to half clock for ~3us after ANY idle gap, so the emission is
built to keep the PE stream dependency-free:
  - Each head's second query superchunk (q in [1024,2048)) is split into the
    off-diagonal RECTANGLE (k < 1024, no mask) and the diagonal TRIANGLE
    (k >= 1024). Rectangles run in era A where the projection quarters 2-3
    provide abundant PE filler to absorb exp()'s ACT-time surplus; their
    unnormalized PSUM partials (plus denominator row) evacuate to SBUF in
    bf16 and are re-injected into era B's PSUM with cheap identity matmuls.
    Era B (triangles only) is then nearly PE/ACT balanced and the output
    projection of rows t < 1024 fills its small remaining deficit.
  - Attention PSUM pairs are released by plain DVE evacuation copies only;
    softmax normalization runs later from SBUF as pump-schedulable filler
    units, so no PE instruction ever waits on a DVE->PE->DVE chain.
  - A virtual-clock model of the PE and ACT streams decides when to pump
    filler between attention steps.
Inputs are host-preswizzled to per-partition-contiguous layouts; later
input DMAs are triggered behind scalar-queue anchor ops so the first-needed
tensors get the full HBM bandwidth (SDMA round-robins packets across all
active queues).
"""

import sys
import types
from collections import deque

import numpy as np

B, T, C, H, D = 4, 2048, 1024, 16, 64
HG = 8            # heads per core
CG = HG * D       # 512 channels per group
NCORES = 8
PE_NS = 1.0 / 2.4      # ns per PE cycle (full p-state)
ACT_NS = 1.0 / 1.2     # ns per ACT lane-cycle
ACT_FIX = 215.0        # measured fixed overhead per ACTIVATE


def _register_ntff_hook():
    """Register the axon NTFF profile hook if the image's antenv lacks it."""
    try:
        import antenv
        if getattr(antenv, "axon_hooks", None) is not None:
            return
        from trn_agent_boot.trn_boot import _ntff_profile_via_ctypes
        hook = _ntff_profile_via_ctypes("/opt/axon/libaxon_pjrt.so")
        mod = types.ModuleType("antenv.axon_hooks")
        mod._hook = hook
        mod.get_axon_ntff_profile_hook = lambda: mod._hook
        mod.set_axon_ntff_profile_hook = lambda h: setattr(mod, "_hook", h)
        sys.modules["antenv.axon_hooks"] = mod
        antenv.axon_hooks = mod
    except Exception:
        pass


_NC_CACHE = {}


def _build():
    import concourse.bacc as bacc
    import concourse.mybir as mybir
    import concourse.tile as tile
    from contextlib import ExitStack

    F32 = mybir.dt.float32
    BF16 = mybir.dt.bfloat16
    ADD = mybir.AluOpType.add
    MUL = mybir.AluOpType.mult
    EXP = mybir.ActivationFunctionType.Exp
    COPY = mybir.ActivationFunctionType.Copy

    nc = bacc.Bacc(None, target_bir_lowering=False, debug=False)
    xq_d = [nc.dram_tensor(f"xq{q}", [128, 8, 512], BF16, kind="ExternalInput")
            for q in range(4)]
    wqk_d = nc.dram_tensor("wqk", [128, 8, 1024], BF16, kind="ExternalInput")
    wv_d = nc.dram_tensor("wv", [128, 8, 512], BF16, kind="ExternalInput")
    wp_d = nc.dram_tensor("wp", [128, 4, 1024], BF16, kind="ExternalInput")
    bqk_d = nc.dram_tensor("bqk", [128, 8], F32, kind="ExternalInput")
    bv_d = nc.dram_tensor("bv", [1, 512], BF16, kind="ExternalInput")
    cst_d = nc.dram_tensor("cst", [128, 256], BF16, kind="ExternalInput")
    out_d = nc.dram_tensor("out", [T, C], F32, kind="ExternalOutput")

    with tile.TileContext(nc) as tc, ExitStack() as ctx:
        pers = ctx.enter_context(tc.tile_pool(name="pers", bufs=1))

        # Head-pair packed qT/kT: pair tile hp holds head 2hp in partitions
        # 0-63 and head 2hp+1 in 64-127, both in [d, t] layout.
        qT = [pers.tile([128, T], BF16, name=f"qT{i}") for i in range(4)]
        kT = [pers.tile([128, T], BF16, name=f"kT{i}") for i in range(4)]
        # v_aug[p, j, h, 0:64] = v[t=j*128+p, h*64+d]; [..., 64] = 1.0
        v_aug = pers.tile([128, 16, HG, 65], BF16, name="v_aug")
        yT = [pers.tile([128, T], BF16, name=f"yT{i}") for i in range(4)]
        # Unnormalized attention partials (64 y rows + denominator row 64).
        yua = [pers.tile([65, 1024], BF16, name=f"yua{h}") for h in range(HG)]
        yur = [pers.tile([65, 1024], BF16, name=f"yur{h}") for h in range(HG)]
        yub = yua   # era-B triangles reuse the A-triangle tiles (their
        # norm-A reads complete before era B writes them)
        cst = pers.tile([128, 256], BF16, name="cst")
        ones_q = pers.tile([1, 512], BF16, name="ones_q")
        ones65 = pers.tile([65, 64], BF16, name="ones65")
        bqk_sb = pers.tile([128, 8], F32, name="bqk_sb")
        bv_sb = pers.tile([1, 512], BF16, name="bv_sb")
        wp_sb = pers.tile([128, 4, 1024], BF16, name="wp_sb")
        anchor = pers.tile([1, 16], F32, name="anchor")

        utri_mask = cst[:, 0:128]
        id128 = cst[:, 128:256]

        att_pool = ctx.enter_context(tc.tile_pool(name="att_pool", bufs=6))
        nrm_pool = ctx.enter_context(tc.tile_pool(name="nrm_pool", bufs=2))
        out_pool = ctx.enter_context(tc.tile_pool(name="out_pool", bufs=3))
        ps_s_pool = ctx.enter_context(
            tc.tile_pool(name="ps_s_pool", bufs=2, space="PSUM"))
        ps_y_pool = ctx.enter_context(
            tc.tile_pool(name="ps_y_pool", bufs=2, space="PSUM"))
        aux_pool = ctx.enter_context(
            tc.tile_pool(name="aux_pool", bufs=2, space="PSUM"))

        # Phase-1 pools (allocated last, released mid-program in reverse).
        wqk_pool = tc.alloc_tile_pool(name="wqk_pool", bufs=1)
        wv_pool = tc.alloc_tile_pool(name="wv_pool", bufs=1)
        xq_pool = tc.alloc_tile_pool(name="xq_pool", bufs=1)
        wqk_sb = wqk_pool.tile([128, 8, 1024], BF16, name="wqk_sb")
        wv_sb = wv_pool.tile([128, 8, 512], BF16, name="wv_sb")
        xq = [xq_pool.tile([128, 8, 512], BF16, name=f"xq{q}") for q in range(4)]

        # bf16 constants staged via f32 memset + rounding copies.
        stage = pers.tile([128, 512], F32, name="stage")
        nc.vector.memset(stage[:], 1.0)
        nc.vector.tensor_copy(ones_q[:], stage[0:1, :])
        nc.vector.tensor_copy(ones65[:], stage[0:65, 0:64])
        nc.vector.tensor_copy(
            v_aug[:, :, :, 64:65],
            stage[:, 0:128].rearrange("p (j h) -> p j h", j=16))

        # Startup DMAs: only the immediately-needed tensors at t0 (SDMA
        # round-robins packets across active queues, so fewer active queues
        # means the first-needed tensors finish sooner); the rest trigger
        # behind scalar-queue anchor ops that depend on early compute.
        nc.scalar.dma_start(bqk_sb[:], bqk_d.ap()[:])
        nc.scalar.dma_start(bv_sb[:], bv_d.ap()[:])
        nc.scalar.dma_start(cst[:], cst_d.ap()[:])
        nc.scalar.dma_start(wv_sb[:], wv_d.ap()[:])
        nc.sync.dma_start(xq[0][:], xq_d[0].ap()[:])
        nc.sync.dma_start(xq[1][:], xq_d[1].ap()[:])

        def late_dmas(stage_no):
            if stage_no == 0:
                nc.scalar.activation(anchor[:], v_aug[0:1, 0, 0, 0:16], COPY)
                nc.scalar.dma_start(wqk_sb[:], wqk_d.ap()[:])
            else:
                nc.scalar.activation(anchor[:], v_aug[0:1, 4, 0, 0:16], COPY)
                nc.scalar.dma_start(xq[2][:], xq_d[2].ap()[:])
                nc.scalar.dma_start(xq[3][:], xq_d[3].ap()[:])
                nc.scalar.dma_start(wp_sb[:], wp_d.ap()[:])

        # ---------------- virtual clocks + filler pump ----------------
        clk = {"pe": 0.0, "act": 0.0}

        def pe(ns):
            clk["pe"] += ns

        fill_hi = deque()   # high-priority filler (deadline-bound)
        fill_lo = deque()

        def pump(target):
            while clk["pe"] < target:
                if fill_hi:
                    fill_hi.popleft()()
                elif fill_lo:
                    fill_lo.popleft()()
                else:
                    break

        # ---------------- phase 1 units ----------------
        def v_unit(q, tb):
            pv = aux_pool.tile([128, 512], F32, name="pv", tag="aux")
            nc.tensor.matmul(pv[:], ones_q[:, tb * 128:(tb + 1) * 128],
                             bv_sb[:], start=True, stop=False)
            for c in range(8):
                nc.tensor.matmul(
                    pv[:], xq[q][:, c, tb * 128:(tb + 1) * 128],
                    wv_sb[:, c, :], start=False, stop=(c == 7))
            j = q * 4 + tb
            nc.vector.tensor_copy(
                v_aug[:, j, :, 0:64], pv[:].rearrange("p (h d) -> p h d", h=HG))
            pe((8 * 512 + 512) * PE_NS)

        def qk_unit(q, m):
            pqk = aux_pool.tile([128, 512], F32, name="pqk", tag="aux")
            for c in range(8):
                nc.tensor.matmul(
                    pqk[:], wqk_sb[:, c, m * 128:(m + 1) * 128],
                    xq[q][:, c, :], start=(c == 0), stop=(c == 7))
            dst = qT[m] if m < 4 else kT[m - 4]
            nc.vector.tensor_scalar(
                out=dst[:, q * 512:(q + 1) * 512], in0=pqk[:],
                scalar1=bqk_sb[:, m:m + 1], scalar2=None, op0=ADD)
            pe(8 * 512 * PE_NS)

        # ---------------- attention steps ----------------
        # Unit kinds: 'A' = q<1024 triangle (j 0-7), 'R' = q>=1024 rectangle
        # (k<1024, j 0-7), 'B' = q>=1024 diagonal triangle (j 8-15).
        KIND = {
            "A": dict(c2=0, js=range(0, 8), stop0=3, stop1=7),
            "R": dict(c2=1, js=range(0, 8), stop0=7, stop1=7),
            "B": dict(c2=1, js=range(8, 16), stop0=11, stop1=15),
        }

        def qk_step(h, c2, j, ps_s):
            hp, hh = h // 2, h % 2
            part = slice(64 * hh, 64 * (hh + 1))
            q0 = 1024 * c2
            dead = max(0, (j - 8 * c2) * 128)
            diag = j >= 8 * c2
            kb = kT[hp][part, j * 128:(j + 1) * 128]
            if dead < 512:
                nc.tensor.matmul(ps_s[:, dead:512], kb,
                                 qT[hp][part, q0 + dead:q0 + 512],
                                 start=True, stop=not diag)
                pe((512 - dead) * PE_NS)
                if diag:
                    nc.tensor.matmul(ps_s[:, dead:dead + 128], utri_mask,
                                     id128, start=False, stop=True,
                                     skip_group_check=True)
                    pe(128 * PE_NS)
                nc.tensor.matmul(ps_s[:, 512:1024], kb,
                                 qT[hp][part, q0 + 512:q0 + 1024],
                                 start=True, stop=True)
                pe(512 * PE_NS)
            else:
                lo = dead
                nc.tensor.matmul(ps_s[:, lo:1024], kb,
                                 qT[hp][part, q0 + lo:q0 + 1024],
                                 start=True, stop=not diag)
                pe((1024 - lo) * PE_NS)
                if diag:
                    nc.tensor.matmul(ps_s[:, lo:lo + 128], utri_mask, id128,
                                     start=False, stop=True,
                                     skip_group_check=True)
                    pe(128 * PE_NS)

        def exp_step(c2, j, ps_s):
            dead = max(0, (j - 8 * c2) * 128)
            att_t = att_pool.tile([128, 1024], BF16, tag="att")
            nc.scalar.activation(att_t[:, dead:1024], ps_s[:, dead:1024],
                                 EXP, scale=0.125)
            clk["act"] = (max(clk["act"], clk["pe"] + 150.0)
                          + (1024 - dead) * ACT_NS + ACT_FIX)
            return att_t

        def av_step(h, kind, j, y0, y1, att_t):
            k = KIND[kind]
            c2, j0 = k["c2"], k["js"][0]
            dead = max(0, (j - 8 * c2) * 128)
            va = v_aug[:, j, h, :]
            if dead < 512:
                nc.tensor.matmul(y0[:, dead:512], va, att_t[:, dead:512],
                                 start=(j == j0), stop=(j == k["stop0"]))
                pe((512 - dead) * PE_NS)
            lo = max(512, dead)
            nc.tensor.matmul(y1[:, lo - 512:512], va, att_t[:, lo:1024],
                             start=(j == j0), stop=(j == k["stop1"]))
            pe((1024 - lo) * PE_NS)

        def inject_rect(h, y0, y1):
            """Add the evacuated rectangle partials into the B-triangle's
            accumulation group via identity matmuls."""
            nc.tensor.matmul(y0[:], id128[0:65, 0:65], yur[h][:, 0:512],
                             start=False, stop=False, skip_group_check=True)
            nc.tensor.matmul(y1[:], id128[0:65, 0:65], yur[h][:, 512:1024],
                             start=False, stop=False, skip_group_check=True)
            pe(1024 * PE_NS)

        def evacuate(dst, y0, y1):
            nc.vector.tensor_copy(dst[:, 0:512], y0[:])
            nc.vector.tensor_copy(dst[:, 512:1024], y1[:])

        # ---------------- softmax normalize (SBUF-based filler) ---------
        def norm_unit(h, cch, src):
            """yT[d, q] = src[d, q] / src[64, q] for one 512-col chunk."""
            sl = slice((cch % 2) * 512, (cch % 2) * 512 + 512)
            ps_b = aux_pool.tile([64, 512], F32, name="ps_b", tag="aux")
            nc.tensor.matmul(ps_b[:], ones65[64:65, :], src[64:65, sl],
                             start=True, stop=True)
            pe(512 * PE_NS)
            inv = nrm_pool.tile([64, 512], F32, tag="inv")
            nc.vector.reciprocal_approx_fast(inv[:], ps_b[:])
            ct, hh = h // 2, h % 2
            dsl = slice(cch * 512, (cch + 1) * 512)
            if hh == 0:
                nc.vector.tensor_tensor(
                    out=yT[ct][0:64, dsl], in0=src[0:64, sl], in1=inv[:],
                    op=MUL)
            else:
                ystg = nrm_pool.tile([64, 512], BF16, tag="ystg")
                nc.vector.tensor_tensor(
                    out=ystg[:], in0=src[0:64, sl], in1=inv[:], op=MUL)
                nc.sync.dma_start(yT[ct][64:128, dsl], ystg[:])

        # ---------------- output projection ----------------
        osb = {}

        def proj_unit(tb, ch, on_act=False):
            if ch == 0:
                osb[tb] = out_pool.tile([128, 1024], F32, name="o_sb",
                                        tag="o_sb")
            pp = aux_pool.tile([128, 512], F32, name="pp", tag="aux")
            for ct in range(4):
                nc.tensor.matmul(
                    pp[:], yT[ct][:, tb * 128:(tb + 1) * 128],
                    wp_sb[:, ct, ch * 512:(ch + 1) * 512],
                    start=(ct == 0), stop=(ct == 3))
            pe(4 * 512 * PE_NS)
            dst = osb[tb][:, ch * 512:(ch + 1) * 512]
            if on_act:
                nc.scalar.activation(dst, pp[:], COPY)
            else:
                nc.vector.tensor_copy(dst, pp[:])
            if ch == 1:
                nc.sync.dma_start(
                    out_d.ap()[tb * 128:(tb + 1) * 128, :], osb.pop(tb)[:])

        # ---------------- attention era pipeline ----------------
        def attn_era(units, margin=800.0, on_unit_done=None, pre_unit=None):
            """units: list of (h, kind). Pipelined emission: QK/exp one step
            ahead of AV; filler pumped before each AV."""
            steps = [(h, kind, j)
                     for h, kind in units for j in KIND[kind]["js"]]
            n = len(steps)
            state = {}
            exp_done = {}
            att_of = {}
            for idx in range(n + 1):
                if idx < n:
                    h, kind, j = steps[idx]
                    if j == KIND[kind]["js"][0]:
                        if pre_unit is not None:
                            pre_unit(h, kind)
                        state[(h, kind)] = (
                            ps_y_pool.tile([65, 512], F32, name="ps_y0",
                                           tag="ps_y"),
                            ps_y_pool.tile([65, 512], F32, name="ps_y1",
                                           tag="ps_y"))
                    ps_s = ps_s_pool.tile([128, 1024], F32, name="ps_s",
                                          tag="ps_s")
                    qk_step(h, KIND[kind]["c2"], j, ps_s)
                    att_of[idx] = exp_step(KIND[kind]["c2"], j, ps_s)
                    exp_done[idx] = clk["act"]
                if idx >= 1:
                    ph, pkind, pj = steps[idx - 1]
                    pump(exp_done[idx - 1] + margin)
                    y0, y1 = state[(ph, pkind)]
                    av_step(ph, pkind, pj, y0, y1, att_of.pop(idx - 1))
                    if pkind == "B" and pj == 8:
                        inject_rect(ph, y0, y1)
                    if pj == KIND[pkind]["js"][-1]:
                        dst = {"A": yua, "R": yur, "B": yub}[pkind][ph]
                        evacuate(dst, y0, y1)
                        del state[(ph, pkind)]
                        if on_unit_done is not None:
                            on_unit_done(ph, pkind)

        # ---------------- orchestration ----------------
        # Phase-1 lead: quarters 0-1 straight through.
        for tb in range(4):
            v_unit(0, tb)
        late_dmas(0)
        for tb in range(4):
            v_unit(1, tb)
        late_dmas(1)
        for q in range(2):
            for m in range(8):
                qk_unit(q, m)

        # Era-A filler: qT quarters 2-3 first (needed by the first rectangle
        # at era-A step 8), then v/kT quarters 2-3 (needed by era B), then
        # normalize units as their inputs are evacuated.
        for q in range(2, 4):
            for m in range(4):
                fill_hi.append(lambda q=q, m=m: qk_unit(q, m))
        for q in range(2, 4):
            for tb in range(4):
                fill_lo.append(lambda q=q, tb=tb: v_unit(q, tb))
            for m in range(4, 8):
                fill_lo.append(lambda q=q, m=m: qk_unit(q, m))

        prev_done = []

        def on_unit_done_a(h, kind):
            # Release the previous unit's normalize work (one-unit delay so
            # the evacuation copies are certainly complete when pumped).
            if prev_done:
                p_h, p_kind = prev_done.pop()
                if p_kind == "A":
                    fill_lo.append(lambda: norm_unit(p_h, 0, yua[p_h]))
                    fill_lo.append(lambda: norm_unit(p_h, 1, yua[p_h]))
            if kind != "R":
                prev_done.append((h, kind))

        def pre_unit_a(h, kind):
            # The first rectangle needs qT quarters 2-3 complete.
            if kind == "R" and h == 0:
                while fill_hi:
                    fill_hi.popleft()()

        units_a = []
        for h in range(HG):
            units_a += [(h, "A"), (h, "R")]
        attn_era(units_a, on_unit_done=on_unit_done_a, pre_unit=pre_unit_a)

        # Boundary: drain remaining quarter/normalize filler (era B needs
        # v_aug j 8-15, kT quarters 2-3 and all A-norms), PE-contiguous.
        while fill_hi:
            fill_hi.popleft()()
        while fill_lo:
            fill_lo.popleft()()
        xq_pool.release()
        wv_pool.release()
        wqk_pool.release()

        # Era B: diagonal triangles. Filler = output projection rows
        # t < 1024 plus B-normalize units as heads complete.
        for tb in range(8):
            for ch in range(2):
                fill_lo.append(lambda tb=tb, ch=ch: proj_unit(tb, ch))

        def on_unit_done_b(h, kind):
            if prev_done:
                p_h, _ = prev_done.pop()
                fill_lo.append(lambda: norm_unit(p_h, 2, yub[p_h]))
                fill_lo.append(lambda: norm_unit(p_h, 3, yub[p_h]))
            prev_done.append((h, kind))

        prev_done.clear()
        order_b = [1, 0, 3, 2, 5, 4, 7, 6]
        attn_era([(h, "B") for h in order_b], on_unit_done=on_unit_done_b)
        while fill_lo:
            fill_lo.popleft()()
        while prev_done:
            p_h, _ = prev_done.pop()
            norm_unit(p_h, 2, yub[p_h])
            norm_unit(p_h, 3, yub[p_h])

        # Tail: rows t >= 1024; psum->sbuf copies ride the idle ACT engine.
        for tb in range(8, 16):
            for ch in range(2):
                proj_unit(tb, ch, on_act=True)

    nc.compile()
    return nc


def _get_nc():
    if "nc" not in _NC_CACHE:
        _register_ntff_hook()
        _NC_CACHE["nc"] = _build()
    return _NC_CACHE["nc"]


def kernel(x, w_attn, b_attn, w_proj, b_proj, _run_kwargs=None):
    import ml_dtypes
    from concourse.bass_utils import run_bass_kernel_spmd

    bf16 = ml_dtypes.bfloat16
    x = np.asarray(x, dtype=np.float32)
    w_attn = np.asarray(w_attn, dtype=np.float32)
    b_attn = np.asarray(b_attn, dtype=np.float32)
    w_proj = np.asarray(w_proj, dtype=np.float32)
    b_proj = np.asarray(b_proj, dtype=np.float32)

    cst = np.concatenate(
        [np.triu(np.ones((128, 128), dtype=np.float32), 1) * (-1000.0),
         np.eye(128, dtype=np.float32)], axis=1).astype(bf16)

    nc = _get_nc()
    in_maps = []
    for core in range(NCORES):
        b, g = divmod(core, 2)
        cs = slice(g * CG, (g + 1) * CG)
        xs = np.ascontiguousarray(
            x[b].T.reshape(8, 128, 4, 512).transpose(1, 2, 0, 3)).astype(bf16)
        wqk = np.concatenate(
            [w_attn[:, cs], w_attn[:, C + g * CG: C + (g + 1) * CG]], axis=1)
        bqk = np.concatenate(
            [b_attn[cs], b_attn[C + g * CG: C + (g + 1) * CG]])
        im = {
            "wqk": np.ascontiguousarray(
                wqk.reshape(8, 128, 1024).transpose(1, 0, 2)).astype(bf16),
            "wv": np.ascontiguousarray(
                w_attn[:, 2 * C + g * CG: 2 * C + (g + 1) * CG]
                .reshape(8, 128, 512).transpose(1, 0, 2)).astype(bf16),
            "wp": np.ascontiguousarray(
                w_proj[cs, :].reshape(4, 128, 1024)
                .transpose(1, 0, 2)).astype(bf16),
            "bqk": np.ascontiguousarray(
                bqk.reshape(8, 128).T).astype(np.float32),
            "bv": b_attn[2 * C + g * CG: 2 * C + (g + 1) * CG]
                .reshape(1, 512).astype(bf16),
            "cst": cst,
        }
        for q in range(4):
            im[f"xq{q}"] = np.ascontiguousarray(xs[:, q]).astype(bf16)
        in_maps.append(im)

    res = run_bass_kernel_spmd(nc, in_maps, core_ids=list(range(NCORES)),
                               **(_run_kwargs or {}))
    out = np.empty((B, T, C), dtype=np.float32)
    for b in range(B):
        out[b] = res.results[2 * b]["out"] + res.results[2 * b + 1]["out"] + b_proj
    if _run_kwargs:
        kernel.last_results = res
    return out


# revision 19
# speedup vs baseline: 1.1438x; 1.1438x over previous
"""Causal self-attention (B=4, T=2048, C=1024, H=16, D=64) on 8 TRN2 NeuronCores.

Sharding: 8 cores = 4 batches x 2 head-groups (8 heads each). Each core:
  - QKV projection for its (batch, head-group) column slice of w_attn,
    producing qT/kT in head-pair-packed [d, t] layout (head 2i in partitions
    0-63, head 2i+1 in 64-127 of pair tile i) and v in [t, d].
  - Causal attention in scoresT layout (scores^T[k, q] straight off the PE;
    K=64 matmuls via PE quadrant base-partition addressing; causal mask
    applied by accumulating -1000*triu into the scores PSUM with one extra
    128-col matmul so exp() emits zeros; softmax denominators via an
    appended ones-column on V).
  - Row-sharded output projection -> per-core partial [T, C].
Host sums the two partials per batch and adds b_proj.

All matmul operands are bf16, accumulation in fp32 PSUM. Scheduling: the PE
p-state drops to half clock for ~3us after ANY idle gap, so the emission is
built to keep the PE stream dependency-free:
  - Each head's second query superchunk (q in [1024,2048)) is split into the
    off-diagonal RECTANGLE (k < 1024, no mask) and the diagonal TRIANGLE
    (k >= 1024). Rectangles run in era A where the projection quarters 2-3
    provide abundant PE filler to absorb exp()'s ACT-time surplus; their
    unnormalized PSUM partials (plus denominator row) evacuate to SBUF in
    bf16 and are re-injected into era B's PSUM with cheap identity matmuls.
    Era B (triangles only) is then nearly PE/ACT balanced and the output
    projection of rows t < 1024 fills its small remaining deficit.
  - Attention PSUM pairs are released by plain DVE evacuation copies only;
    softmax normalization runs later from SBUF as pump-schedulable filler
    units, so no PE instruction ever waits on a DVE->PE->DVE chain.
  - A virtual-clock model of the PE and ACT streams decides when to pump
    filler between attention steps.
Inputs are host-preswizzled to per-partition-contiguous layouts; later
input DMAs are triggered behind scalar-queue anchor ops so the first-needed
tensors get the full HBM bandwidth (SDMA round-robins packets across all
active queues).
"""

import sys
import types
from collections import deque

import numpy as np

B, T, C, H, D = 4, 2048, 1024, 16, 64
HG = 8            # heads per core
CG = HG * D       # 512 channels per group
NCORES = 8
PE_NS = 1.0 / 2.4      # ns per PE cycle (full p-state)
ACT_NS = 1.0 / 1.2     # ns per ACT lane-cycle
ACT_FIX = 215.0        # measured fixed overhead per ACTIVATE


def _register_ntff_hook():
    """Register the axon NTFF profile hook if the image's antenv lacks it."""
    try:
        import antenv
        if getattr(antenv, "axon_hooks", None) is not None:
            return
        from trn_agent_boot.trn_boot import _ntff_profile_via_ctypes
        hook = _ntff_profile_via_ctypes("/opt/axon/libaxon_pjrt.so")
        mod = types.ModuleType("antenv.axon_hooks")
        mod._hook = hook
        mod.get_axon_ntff_profile_hook = lambda: mod._hook
        mod.set_axon_ntff_profile_hook = lambda h: setattr(mod, "_hook", h)
        sys.modules["antenv.axon_hooks"] = mod
        antenv.axon_hooks = mod
    except Exception:
        pass


_NC_CACHE = {}


def _build():
    import concourse.bacc as bacc
    import concourse.mybir as mybir
    import concourse.tile as tile
    from contextlib import ExitStack

    F32 = mybir.dt.float32
    BF16 = mybir.dt.bfloat16
    ADD = mybir.AluOpType.add
    MUL = mybir.AluOpType.mult
    EXP = mybir.ActivationFunctionType.Exp
    COPY = mybir.ActivationFunctionType.Copy

    nc = bacc.Bacc(None, target_bir_lowering=False, debug=False)
    xq_d = [nc.dram_tensor(f"xq{q}", [128, 8, 512], BF16, kind="ExternalInput")
            for q in range(4)]
    wqk_d = nc.dram_tensor("wqk", [128, 8, 1024], BF16, kind="ExternalInput")
    wv_d = nc.dram_tensor("wv", [128, 8, 512], BF16, kind="ExternalInput")
    wp_d = nc.dram_tensor("wp", [128, 4, 1024], BF16, kind="ExternalInput")
    bqk_d = nc.dram_tensor("bqk", [128, 8], F32, kind="ExternalInput")
    bv_d = nc.dram_tensor("bv", [1, 512], BF16, kind="ExternalInput")
    cst_d = nc.dram_tensor("cst", [128, 256], BF16, kind="ExternalInput")
    out_d = nc.dram_tensor("out", [T, C], F32, kind="ExternalOutput")

    with tile.TileContext(nc) as tc, ExitStack() as ctx:
        pers = ctx.enter_context(tc.tile_pool(name="pers", bufs=1))

        # Head-pair packed qT/kT: pair tile hp holds head 2hp in partitions
        # 0-63 and head 2hp+1 in 64-127, both in [d, t] layout.
        # All cross-phase tensors are split into disjoint half tiles so
        # the Tile dependency tracker never couples attention reads of one
        # half with filler writes of the other.
        qTl = [pers.tile([128, 1024], BF16, name=f"qTl{i}") for i in range(4)]
        qTh = [pers.tile([128, 1024], BF16, name=f"qTh{i}") for i in range(4)]
        kTl = [pers.tile([128, 1024], BF16, name=f"kTl{i}") for i in range(4)]
        kTh = [pers.tile([128, 1024], BF16, name=f"kTh{i}") for i in range(4)]
        # v*[p, j, h, 0:64] = v[t=j*128+p, h*64+d]; [..., 64] = 1.0
        vgl = pers.tile([128, 8, HG, 65], BF16, name="vgl")
        vgh = pers.tile([128, 8, HG, 65], BF16, name="vgh")
        yTl = [pers.tile([128, 1024], BF16, name=f"yTl{i}") for i in range(4)]
        yTh = [pers.tile([128, 1024], BF16, name=f"yTh{i}") for i in range(4)]
        # Unnormalized attention partials (64 y rows + denominator row 64).
        yua = [pers.tile([65, 1024], BF16, name=f"yua{h}") for h in range(HG)]
        yur = [pers.tile([65, 1024], BF16, name=f"yur{h}") for h in range(HG)]
        yub = yua   # era-B triangles reuse the A-triangle tiles (their
        # norm-A reads complete before era B writes them)
        cst = pers.tile([128, 256], BF16, name="cst")
        ones_q = pers.tile([1, 512], BF16, name="ones_q")
        ones65 = pers.tile([65, 64], BF16, name="ones65")
        bqk_sb = pers.tile([128, 8], F32, name="bqk_sb")
        bv_sb = pers.tile([1, 512], BF16, name="bv_sb")
        wp_sb = pers.tile([128, 4, 1024], BF16, name="wp_sb")
        anchor = pers.tile([1, 16], F32, name="anchor")

        utri_mask = cst[:, 0:128]
        id128 = cst[:, 128:256]

        att_pool = ctx.enter_context(tc.tile_pool(name="att_pool", bufs=6))
        nrm_pool = ctx.enter_context(tc.tile_pool(name="nrm_pool", bufs=2))
        out_pool = ctx.enter_context(tc.tile_pool(name="out_pool", bufs=3))
        ps_s_pool = ctx.enter_context(
            tc.tile_pool(name="ps_s_pool", bufs=2, space="PSUM"))
        ps_y_pool = ctx.enter_context(
            tc.tile_pool(name="ps_y_pool", bufs=2, space="PSUM"))
        aux_pool = ctx.enter_context(
            tc.tile_pool(name="aux_pool", bufs=2, space="PSUM"))

        # Phase-1 pools (allocated last, released mid-program in reverse).
        wqk_pool = tc.alloc_tile_pool(name="wqk_pool", bufs=1)
        wv_pool = tc.alloc_tile_pool(name="wv_pool", bufs=1)
        xq_pool = tc.alloc_tile_pool(name="xq_pool", bufs=1)
        wqk_sb = wqk_pool.tile([128, 8, 1024], BF16, name="wqk_sb")
        wv_sb = wv_pool.tile([128, 8, 512], BF16, name="wv_sb")
        xq = [xq_pool.tile([128, 8, 512], BF16, name=f"xq{q}") for q in range(4)]

        # bf16 constants staged via f32 memset + rounding copies.
        stage = pers.tile([128, 512], F32, name="stage")
        nc.vector.memset(stage[:], 1.0)
        nc.vector.tensor_copy(ones_q[:], stage[0:1, :])
        nc.vector.tensor_copy(ones65[:], stage[0:65, 0:64])
        nc.vector.tensor_copy(
            vgl[:, :, :, 64:65],
            stage[:, 0:64].rearrange("p (j h) -> p j h", j=8))
        nc.vector.tensor_copy(
            vgh[:, :, :, 64:65],
            stage[:, 0:64].rearrange("p (j h) -> p j h", j=8))

        # Startup DMAs: only the immediately-needed tensors at t0 (SDMA
        # round-robins packets across active queues, so fewer active queues
        # means the first-needed tensors finish sooner); the rest trigger
        # behind scalar-queue anchor ops that depend on early compute.
        nc.scalar.dma_start(bqk_sb[:], bqk_d.ap()[:])
        nc.scalar.dma_start(bv_sb[:], bv_d.ap()[:])
        nc.scalar.dma_start(cst[:], cst_d.ap()[:])
        nc.scalar.dma_start(wv_sb[:], wv_d.ap()[:])
        nc.sync.dma_start(xq[0][:], xq_d[0].ap()[:])
        nc.sync.dma_start(xq[1][:], xq_d[1].ap()[:])

        def late_dmas(stage_no):
            if stage_no == 0:
                nc.scalar.activation(anchor[:], vgl[0:1, 0, 0, 0:16], COPY)
                nc.scalar.dma_start(wqk_sb[:], wqk_d.ap()[:])
            else:
                nc.scalar.activation(anchor[:], vgl[0:1, 4, 0, 0:16], COPY)
                nc.scalar.dma_start(xq[2][:], xq_d[2].ap()[:])
                nc.scalar.dma_start(xq[3][:], xq_d[3].ap()[:])
                nc.scalar.dma_start(wp_sb[:], wp_d.ap()[:])

        # ---------------- virtual clocks + filler pump ----------------
        clk = {"pe": 0.0, "act": 0.0}

        def pe(ns):
            clk["pe"] += ns

        fill_hi = deque()   # high-priority filler (deadline-bound)
        fill_lo = deque()

        def pump(target):
            while clk["pe"] < target:
                if fill_hi:
                    fill_hi.popleft()()
                elif fill_lo:
                    fill_lo.popleft()()
                else:
                    break

        # ------- phase 1 units (emitted as per-matmul thunks) -------
        def v_unit_thunks(q, tb):
            st = {}

            def first():
                st["pv"] = aux_pool.tile([128, 512], F32, name="pv",
                                         tag="aux")
                nc.tensor.matmul(st["pv"][:],
                                 ones_q[:, tb * 128:(tb + 1) * 128],
                                 bv_sb[:], start=True, stop=False)
                pe(512 * PE_NS)

            def mid(c):
                nc.tensor.matmul(
                    st["pv"][:], xq[q][:, c, tb * 128:(tb + 1) * 128],
                    wv_sb[:, c, :], start=False, stop=(c == 7))
                pe(512 * PE_NS)

            def last():
                j = q * 4 + tb
                vg = vgl if j < 8 else vgh
                nc.vector.tensor_copy(
                    vg[:, j % 8, :, 0:64],
                    st["pv"][:].rearrange("p (h d) -> p h d", h=HG))
            return ([first] + [lambda c=c: mid(c) for c in range(8)]
                    + [last])

        def qk_unit_thunks(q, m):
            st = {}

            def mid(c):
                if c == 0:
                    st["pqk"] = aux_pool.tile([128, 512], F32, name="pqk",
                                              tag="aux")
                nc.tensor.matmul(
                    st["pqk"][:], wqk_sb[:, c, m * 128:(m + 1) * 128],
                    xq[q][:, c, :], start=(c == 0), stop=(c == 7))
                pe(512 * PE_NS)

            def last():
                half = [qTl, qTh, kTl, kTh][(m >= 4) * 2 + (q >= 2)]
                dst = half[m % 4]
                nc.vector.tensor_scalar(
                    out=dst[:, (q % 2) * 512:(q % 2) * 512 + 512],
                    in0=st["pqk"][:],
                    scalar1=bqk_sb[:, m:m + 1], scalar2=None, op0=ADD)
            return [lambda c=c: mid(c) for c in range(8)] + [last]

        def v_unit(q, tb):
            for t in v_unit_thunks(q, tb):
                t()

        def qk_unit(q, m):
            for t in qk_unit_thunks(q, m):
                t()

        # ---------------- attention steps ----------------
        # Unit kinds: 'A' = q<1024 triangle (j 0-7), 'R' = q>=1024 rectangle
        # (k<1024, j 0-7), 'B' = q>=1024 diagonal triangle (j 8-15).
        KIND = {
            "A": dict(c2=0, js=range(0, 8), stop0=3, stop1=7),
            "R": dict(c2=1, js=range(0, 8), stop0=7, stop1=7),
            "B": dict(c2=1, js=range(8, 16), stop0=11, stop1=15),
        }

        def qk_step(h, c2, j, ps_s):
            hp, hh = h // 2, h % 2
            part = slice(64 * hh, 64 * (hh + 1))
            dead = max(0, (j - 8 * c2) * 128)
            diag = j >= 8 * c2
            qTt = (qTl if c2 == 0 else qTh)[hp]
            kTt = (kTl if j < 8 else kTh)[hp]
            kb = kTt[part, (j % 8) * 128:(j % 8) * 128 + 128]
            if dead < 512:
                nc.tensor.matmul(ps_s[:, dead:512], kb,
                                 qTt[part, dead:512],
                                 start=True, stop=not diag)
                pe((512 - dead) * PE_NS)
                if diag:
                    nc.tensor.matmul(ps_s[:, dead:dead + 128], utri_mask,
                                     id128, start=False, stop=True,
                                     skip_group_check=True)
                    pe(128 * PE_NS)
                nc.tensor.matmul(ps_s[:, 512:1024], kb,
                                 qTt[part, 512:1024],
                                 start=True, stop=True)
                pe(512 * PE_NS)
            else:
                lo = dead
                nc.tensor.matmul(ps_s[:, lo:1024], kb,
                                 qTt[part, lo:1024],
                                 start=True, stop=not diag)
                pe((1024 - lo) * PE_NS)
                if diag:
                    nc.tensor.matmul(ps_s[:, lo:lo + 128], utri_mask, id128,
                                     start=False, stop=True,
                                     skip_group_check=True)
                    pe(128 * PE_NS)

        def exp_step(c2, j, ps_s):
            dead = max(0, (j - 8 * c2) * 128)
            att_t = att_pool.tile([128, 1024], BF16, tag="att")
            nc.scalar.activation(att_t[:, dead:1024], ps_s[:, dead:1024],
                                 EXP, scale=0.125)
            clk["act"] = (max(clk["act"], clk["pe"] + 150.0)
                          + (1024 - dead) * ACT_NS + ACT_FIX)
            return att_t

        def av_step(h, kind, j, y0, y1, att_t):
            k = KIND[kind]
            c2, j0 = k["c2"], k["js"][0]
            dead = max(0, (j - 8 * c2) * 128)
            va = (vgl if j < 8 else vgh)[:, j % 8, h, :]
            if dead < 512:
                nc.tensor.matmul(y0[:, dead:512], va, att_t[:, dead:512],
                                 start=(j == j0), stop=(j == k["stop0"]))
                pe((512 - dead) * PE_NS)
            lo = max(512, dead)
            nc.tensor.matmul(y1[:, lo - 512:512], va, att_t[:, lo:1024],
                             start=(j == j0), stop=(j == k["stop1"]))
            pe((1024 - lo) * PE_NS)

        def inject_rect(h, y0, y1):
            """Add the evacuated rectangle partials into the B-triangle's
            accumulation group via identity matmuls."""
            nc.tensor.matmul(y0[:], id128[0:65, 0:65], yur[h][:, 0:512],
                             start=False, stop=False, skip_group_check=True)
            nc.tensor.matmul(y1[:], id128[0:65, 0:65], yur[h][:, 512:1024],
                             start=False, stop=False, skip_group_check=True)
            pe(1024 * PE_NS)

        def evacuate(dst, y0, y1):
            nc.vector.tensor_copy(dst[:, 0:512], y0[:])
            nc.vector.tensor_copy(dst[:, 512:1024], y1[:])

        # ---------------- softmax normalize (SBUF-based filler) ---------
        def norm_unit(h, cch, src):
            """yT[d, q] = src[d, q] / src[64, q] for one 512-col chunk."""
            sl = slice((cch % 2) * 512, (cch % 2) * 512 + 512)
            ps_b = aux_pool.tile([64, 512], F32, name="ps_b", tag="aux")
            nc.tensor.matmul(ps_b[:], ones65[64:65, :], src[64:65, sl],
                             start=True, stop=True)
            pe(512 * PE_NS)
            inv = nrm_pool.tile([64, 512], F32, tag="inv")
            nc.vector.reciprocal_approx_fast(inv[:], ps_b[:])
            ct, hh = h // 2, h % 2
            yTt = (yTl if cch < 2 else yTh)[ct]
            dsl = slice((cch % 2) * 512, (cch % 2) * 512 + 512)
            if hh == 0:
                nc.vector.tensor_tensor(
                    out=yTt[0:64, dsl], in0=src[0:64, sl], in1=inv[:],
                    op=MUL)
            else:
                ystg = nrm_pool.tile([64, 512], BF16, tag="ystg")
                nc.vector.tensor_tensor(
                    out=ystg[:], in0=src[0:64, sl], in1=inv[:], op=MUL)
                nc.sync.dma_start(yTt[64:128, dsl], ystg[:])

        # ---------------- output projection ----------------
        osb = {}

        def proj_unit_thunks(tb, ch, on_act=False):
            st = {}
            yTt = yTl if tb < 8 else yTh

            def mid(ct):
                if ct == 0:
                    if ch == 0:
                        osb[tb] = out_pool.tile([128, 1024], F32,
                                                name="o_sb", tag="o_sb")
                    st["pp"] = aux_pool.tile([128, 512], F32, name="pp",
                                             tag="aux")
                nc.tensor.matmul(
                    st["pp"][:], yTt[ct][:, (tb % 8) * 128:(tb % 8) * 128 + 128],
                    wp_sb[:, ct, ch * 512:(ch + 1) * 512],
                    start=(ct == 0), stop=(ct == 3))
                pe(512 * PE_NS)

            def last():
                dst = osb[tb][:, ch * 512:(ch + 1) * 512]
                if on_act:
                    nc.scalar.activation(dst, st["pp"][:], COPY)
                else:
                    nc.vector.tensor_copy(dst, st["pp"][:])
                if ch == 1:
                    nc.sync.dma_start(
                        out_d.ap()[tb * 128:(tb + 1) * 128, :],
                        osb.pop(tb)[:])
            return [lambda ct=ct: mid(ct) for ct in range(4)] + [last]

        def proj_unit(tb, ch, on_act=False):
            for t in proj_unit_thunks(tb, ch, on_act):
                t()

        # ---------------- attention era pipeline ----------------
        def attn_era(units, margin=800.0, on_unit_done=None, pre_unit=None):
            """units: list of (h, kind). Pipelined emission: QK/exp one step
            ahead of AV; filler pumped before each AV."""
            steps = [(h, kind, j)
                     for h, kind in units for j in KIND[kind]["js"]]
            n = len(steps)
            state = {}
            exp_done = {}
            att_of = {}
            for idx in range(n + 1):
                if idx < n:
                    h, kind, j = steps[idx]
                    if j == KIND[kind]["js"][0]:
                        if pre_unit is not None:
                            pre_unit(h, kind)
                        state[(h, kind)] = (
                            ps_y_pool.tile([65, 512], F32, name="ps_y0",
                                           tag="ps_y"),
                            ps_y_pool.tile([65, 512], F32, name="ps_y1",
                                           tag="ps_y"))
                    ps_s = ps_s_pool.tile([128, 1024], F32, name="ps_s",
                                          tag="ps_s")
                    qk_step(h, KIND[kind]["c2"], j, ps_s)
                    att_of[idx] = exp_step(KIND[kind]["c2"], j, ps_s)
                    exp_done[idx] = clk["act"]
                if idx >= 1:
                    ph, pkind, pj = steps[idx - 1]
                    pump(exp_done[idx - 1] + margin)
                    y0, y1 = state[(ph, pkind)]
                    av_step(ph, pkind, pj, y0, y1, att_of.pop(idx - 1))
                    if pkind == "B" and pj == 8:
                        inject_rect(ph, y0, y1)
                    if pj == KIND[pkind]["js"][-1]:
                        dst = {"A": yua, "R": yur, "B": yub}[pkind][ph]
                        evacuate(dst, y0, y1)
                        del state[(ph, pkind)]
                        if on_unit_done is not None:
                            on_unit_done(ph, pkind)

        # ---------------- orchestration ----------------
        # Phase-1 lead: quarters 0-1 straight through.
        for tb in range(4):
            v_unit(0, tb)
        late_dmas(0)
        for tb in range(4):
            v_unit(1, tb)
        late_dmas(1)
        for q in range(2):
            for m in range(8):
                qk_unit(q, m)

        # Era-A filler: qT quarters 2-3 first (needed by the first rectangle
        # at era-A step 8), then v/kT quarters 2-3 (needed by era B), then
        # normalize units as their inputs are evacuated.
        for q in range(2, 4):
            for m in range(4):
                fill_hi.extend(qk_unit_thunks(q, m))
        for q in range(2, 4):
            for tb in range(4):
                fill_lo.extend(v_unit_thunks(q, tb))
            for m in range(4, 8):
                fill_lo.extend(qk_unit_thunks(q, m))

        prev_done = []

        def on_unit_done_a(h, kind):
            # Release the previous unit's normalize work (one-unit delay so
            # the evacuation copies are certainly complete when pumped).
            if prev_done:
                p_h, p_kind = prev_done.pop()
                if p_kind == "A":
                    fill_lo.append(lambda: norm_unit(p_h, 0, yua[p_h]))
                    fill_lo.append(lambda: norm_unit(p_h, 1, yua[p_h]))
            if kind != "R":
                prev_done.append((h, kind))

        def pre_unit_a(h, kind):
            # The first rectangle needs qT quarters 2-3 complete.
            if kind == "R" and h == 0:
                while fill_hi:
                    fill_hi.popleft()()

        units_a = []
        for h in range(HG):
            units_a += [(h, "A"), (h, "R")]
        attn_era(units_a, on_unit_done=on_unit_done_a, pre_unit=pre_unit_a)

        # Boundary: drain remaining quarter/normalize filler (era B needs
        # v_aug j 8-15, kT quarters 2-3 and all A-norms), PE-contiguous.
        while fill_hi:
            fill_hi.popleft()()
        while fill_lo:
            fill_lo.popleft()()
        xq_pool.release()
        wv_pool.release()
        wqk_pool.release()

        # Era B: diagonal triangles. Filler = output projection rows
        # t < 1024 plus B-normalize units as heads complete.
        for tb in range(8):
            for ch in range(2):
                fill_lo.extend(proj_unit_thunks(tb, ch))

        def on_unit_done_b(h, kind):
            if prev_done:
                p_h, _ = prev_done.pop()
                fill_lo.append(lambda: norm_unit(p_h, 2, yub[p_h]))
                fill_lo.append(lambda: norm_unit(p_h, 3, yub[p_h]))
            prev_done.append((h, kind))

        prev_done.clear()
        order_b = [1, 0, 3, 2, 5, 4, 7, 6]
        attn_era([(h, "B") for h in order_b], on_unit_done=on_unit_done_b)
        while fill_lo:
            fill_lo.popleft()()
        while prev_done:
            p_h, _ = prev_done.pop()
            norm_unit(p_h, 2, yub[p_h])
            norm_unit(p_h, 3, yub[p_h])

        # Tail: rows t >= 1024; psum->sbuf copies ride the idle ACT engine.
        for tb in range(8, 16):
            for ch in range(2):
                proj_unit(tb, ch, on_act=True)

    nc.compile()
    return nc


def _get_nc():
    if "nc" not in _NC_CACHE:
        _register_ntff_hook()
        _NC_CACHE["nc"] = _build()
    return _NC_CACHE["nc"]


def kernel(x, w_attn, b_attn, w_proj, b_proj, _run_kwargs=None):
    import ml_dtypes
    from concourse.bass_utils import run_bass_kernel_spmd

    bf16 = ml_dtypes.bfloat16
    x = np.asarray(x, dtype=np.float32)
    w_attn = np.asarray(w_attn, dtype=np.float32)
    b_attn = np.asarray(b_attn, dtype=np.float32)
    w_proj = np.asarray(w_proj, dtype=np.float32)
    b_proj = np.asarray(b_proj, dtype=np.float32)

    cst = np.concatenate(
        [np.triu(np.ones((128, 128), dtype=np.float32), 1) * (-1000.0),
         np.eye(128, dtype=np.float32)], axis=1).astype(bf16)

    nc = _get_nc()
    in_maps = []
    for core in range(NCORES):
        b, g = divmod(core, 2)
        cs = slice(g * CG, (g + 1) * CG)
        xs = np.ascontiguousarray(
            x[b].T.reshape(8, 128, 4, 512).transpose(1, 2, 0, 3)).astype(bf16)
        wqk = np.concatenate(
            [w_attn[:, cs], w_attn[:, C + g * CG: C + (g + 1) * CG]], axis=1)
        bqk = np.concatenate(
            [b_attn[cs], b_attn[C + g * CG: C + (g + 1) * CG]])
        im = {
            "wqk": np.ascontiguousarray(
                wqk.reshape(8, 128, 1024).transpose(1, 0, 2)).astype(bf16),
            "wv": np.ascontiguousarray(
                w_attn[:, 2 * C + g * CG: 2 * C + (g + 1) * CG]
                .reshape(8, 128, 512).transpose(1, 0, 2)).astype(bf16),
            "wp": np.ascontiguousarray(
                w_proj[cs, :].reshape(4, 128, 1024)
                .transpose(1, 0, 2)).astype(bf16),
            "bqk": np.ascontiguousarray(
                bqk.reshape(8, 128).T).astype(np.float32),
            "bv": b_attn[2 * C + g * CG: 2 * C + (g + 1) * CG]
                .reshape(1, 512).astype(bf16),
            "cst": cst,
        }
        for q in range(4):
            im[f"xq{q}"] = np.ascontiguousarray(xs[:, q]).astype(bf16)
        in_maps.append(im)

    res = run_bass_kernel_spmd(nc, in_maps, core_ids=list(range(NCORES)),
                               **(_run_kwargs or {}))
    out = np.empty((B, T, C), dtype=np.float32)
    for b in range(B):
        out[b] = res.results[2 * b]["out"] + res.results[2 * b + 1]["out"] + b_proj
    if _run_kwargs:
        kernel.last_results = res
    return out


# revision 20
# speedup vs baseline: 1.2201x; 1.0667x over previous
"""Causal self-attention (B=4, T=2048, C=1024, H=16, D=64) on 8 TRN2 NeuronCores.

Sharding: 8 cores = 4 batches x 2 head-groups (8 heads each). Each core:
  - QKV projection for its (batch, head-group) column slice of w_attn,
    producing qT/kT in head-pair-packed [d, t] layout (head 2i in partitions
    0-63, head 2i+1 in 64-127 of pair tile i) and v in [t, d].
  - Causal attention in scoresT layout (scores^T[k, q] straight off the PE;
    K=64 matmuls via PE quadrant base-partition addressing; causal mask
    applied by accumulating -1000*triu into the scores PSUM with one extra
    128-col matmul so exp() emits zeros; softmax denominators via an
    appended ones-column on V).
  - Row-sharded output projection -> per-core partial [T, C].
Host sums the two partials per batch and adds b_proj.

All matmul operands are bf16, accumulation in fp32 PSUM. Scheduling: the PE
p-state drops to half clock for ~3us after ANY idle gap, so the emission is
built to keep the PE stream dependency-free:
  - Each head's second query superchunk (q in [1024,2048)) is split into the
    off-diagonal RECTANGLE (k < 1024, no mask) and the diagonal TRIANGLE
    (k >= 1024). Rectangles run in era A where the projection quarters 2-3
    provide abundant PE filler to absorb exp()'s ACT-time surplus; their
    unnormalized PSUM partials (plus denominator row) evacuate to SBUF in
    bf16 and are re-injected into era B's PSUM with cheap identity matmuls.
    Era B (triangles only) is then nearly PE/ACT balanced and the output
    projection of rows t < 1024 fills its small remaining deficit.
  - Attention PSUM pairs are released by plain DVE evacuation copies only;
    softmax normalization runs later from SBUF as pump-schedulable filler
    units, so no PE instruction ever waits on a DVE->PE->DVE chain.
  - A virtual-clock model of the PE and ACT streams decides when to pump
    filler between attention steps.
Inputs are host-preswizzled to per-partition-contiguous layouts; later
input DMAs are triggered behind scalar-queue anchor ops so the first-needed
tensors get the full HBM bandwidth (SDMA round-robins packets across all
active queues).
"""

import sys
import types
from collections import deque

import numpy as np

B, T, C, H, D = 4, 2048, 1024, 16, 64
HG = 8            # heads per core
CG = HG * D       # 512 channels per group
NCORES = 8
PE_NS = 1.0 / 2.4      # ns per PE cycle (full p-state)
MMF = 50.0             # per-matmul fixed overhead (issue + sem lag)
ACT_NS = 1.0 / 1.2     # ns per ACT lane-cycle
ACT_FIX = 215.0        # measured fixed overhead per ACTIVATE


def _register_ntff_hook():
    """Register the axon NTFF profile hook if the image's antenv lacks it."""
    try:
        import antenv
        if getattr(antenv, "axon_hooks", None) is not None:
            return
        from trn_agent_boot.trn_boot import _ntff_profile_via_ctypes
        hook = _ntff_profile_via_ctypes("/opt/axon/libaxon_pjrt.so")
        mod = types.ModuleType("antenv.axon_hooks")
        mod._hook = hook
        mod.get_axon_ntff_profile_hook = lambda: mod._hook
        mod.set_axon_ntff_profile_hook = lambda h: setattr(mod, "_hook", h)
        sys.modules["antenv.axon_hooks"] = mod
        antenv.axon_hooks = mod
    except Exception:
        pass


_NC_CACHE = {}


def _build():
    import concourse.bacc as bacc
    import concourse.mybir as mybir
    import concourse.tile as tile
    from contextlib import ExitStack

    F32 = mybir.dt.float32
    BF16 = mybir.dt.bfloat16
    ADD = mybir.AluOpType.add
    MUL = mybir.AluOpType.mult
    EXP = mybir.ActivationFunctionType.Exp
    COPY = mybir.ActivationFunctionType.Copy

    nc = bacc.Bacc(None, target_bir_lowering=False, debug=False)
    xq_d = [nc.dram_tensor(f"xq{q}", [128, 8, 512], BF16, kind="ExternalInput")
            for q in range(4)]
    wqk_d = nc.dram_tensor("wqk", [128, 8, 1024], BF16, kind="ExternalInput")
    wv_d = nc.dram_tensor("wv", [128, 8, 512], BF16, kind="ExternalInput")
    wp_d = nc.dram_tensor("wp", [128, 4, 1024], BF16, kind="ExternalInput")
    bqk_d = nc.dram_tensor("bqk", [128, 8], F32, kind="ExternalInput")
    bv_d = nc.dram_tensor("bv", [1, 512], BF16, kind="ExternalInput")
    cst_d = nc.dram_tensor("cst", [128, 256], BF16, kind="ExternalInput")
    out_d = nc.dram_tensor("out", [T, C], F32, kind="ExternalOutput")

    with tile.TileContext(nc) as tc, ExitStack() as ctx:
        pers = ctx.enter_context(tc.tile_pool(name="pers", bufs=1))

        # Head-pair packed qT/kT: pair tile hp holds head 2hp in partitions
        # 0-63 and head 2hp+1 in 64-127, both in [d, t] layout.
        # All cross-phase tensors are split into disjoint half tiles so
        # the Tile dependency tracker never couples attention reads of one
        # half with filler writes of the other.
        qTl = [pers.tile([128, 1024], BF16, name=f"qTl{i}") for i in range(4)]
        qTh = [pers.tile([128, 1024], BF16, name=f"qTh{i}") for i in range(4)]
        kTl = [pers.tile([128, 1024], BF16, name=f"kTl{i}") for i in range(4)]
        kTh = [pers.tile([128, 1024], BF16, name=f"kTh{i}") for i in range(4)]
        # v*[p, j, h, 0:64] = v[t=j*128+p, h*64+d]; [..., 64] = 1.0
        vgl = pers.tile([128, 8, HG, 65], BF16, name="vgl")
        vgh = pers.tile([128, 8, HG, 65], BF16, name="vgh")
        yTl = [pers.tile([128, 1024], BF16, name=f"yTl{i}") for i in range(4)]
        yTh = [pers.tile([128, 1024], BF16, name=f"yTh{i}") for i in range(4)]
        # Unnormalized attention partials (64 y rows + denominator row 64).
        yua = [pers.tile([65, 1024], BF16, name=f"yua{h}") for h in range(HG)]
        yur = [pers.tile([65, 1024], BF16, name=f"yur{h}") for h in range(HG)]
        yub = yua   # era-B triangles reuse the A-triangle tiles (their
        # norm-A reads complete before era B writes them)
        cst = pers.tile([128, 256], BF16, name="cst")
        ones_q = pers.tile([1, 512], BF16, name="ones_q")
        ones65 = pers.tile([65, 64], BF16, name="ones65")
        bqk_sb = pers.tile([128, 8], F32, name="bqk_sb")
        bv_sb = pers.tile([1, 512], BF16, name="bv_sb")
        wp_sb = pers.tile([128, 4, 1024], BF16, name="wp_sb")
        anchor = pers.tile([1, 16], F32, name="anchor")

        utri_mask = cst[:, 0:128]
        id128 = cst[:, 128:256]

        att_pool = ctx.enter_context(tc.tile_pool(name="att_pool", bufs=6))
        nrm_pool = ctx.enter_context(tc.tile_pool(name="nrm_pool", bufs=2))
        out_pool = ctx.enter_context(tc.tile_pool(name="out_pool", bufs=3))
        ps_s_pool = ctx.enter_context(
            tc.tile_pool(name="ps_s_pool", bufs=2, space="PSUM"))
        ps_y_pool = ctx.enter_context(
            tc.tile_pool(name="ps_y_pool", bufs=2, space="PSUM"))
        aux_pool = ctx.enter_context(
            tc.tile_pool(name="aux_pool", bufs=2, space="PSUM"))

        # Phase-1 pools (allocated last, released mid-program in reverse).
        wqk_pool = tc.alloc_tile_pool(name="wqk_pool", bufs=1)
        wv_pool = tc.alloc_tile_pool(name="wv_pool", bufs=1)
        xq_pool = tc.alloc_tile_pool(name="xq_pool", bufs=1)
        wqk_sb = wqk_pool.tile([128, 8, 1024], BF16, name="wqk_sb")
        wv_sb = wv_pool.tile([128, 8, 512], BF16, name="wv_sb")
        xq = [xq_pool.tile([128, 8, 512], BF16, name=f"xq{q}") for q in range(4)]

        # bf16 constants staged via f32 memset + rounding copies.
        stage = pers.tile([128, 512], F32, name="stage")
        nc.vector.memset(stage[:], 1.0)
        nc.vector.tensor_copy(ones_q[:], stage[0:1, :])
        nc.vector.tensor_copy(ones65[:], stage[0:65, 0:64])
        nc.vector.tensor_copy(
            vgl[:, :, :, 64:65],
            stage[:, 0:64].rearrange("p (j h) -> p j h", j=8))
        nc.vector.tensor_copy(
            vgh[:, :, :, 64:65],
            stage[:, 0:64].rearrange("p (j h) -> p j h", j=8))

        # Startup DMAs: only the immediately-needed tensors at t0 (SDMA
        # round-robins packets across active queues, so fewer active queues
        # means the first-needed tensors finish sooner); the rest trigger
        # behind scalar-queue anchor ops that depend on early compute.
        nc.scalar.dma_start(bqk_sb[:], bqk_d.ap()[:])
        nc.scalar.dma_start(bv_sb[:], bv_d.ap()[:])
        nc.scalar.dma_start(cst[:], cst_d.ap()[:])
        nc.scalar.dma_start(wv_sb[:], wv_d.ap()[:])
        nc.sync.dma_start(xq[0][:], xq_d[0].ap()[:])
        nc.sync.dma_start(xq[1][:], xq_d[1].ap()[:])

        def late_dmas(stage_no):
            if stage_no == 0:
                nc.scalar.activation(anchor[:], vgl[0:1, 0, 0, 0:16], COPY)
                nc.scalar.dma_start(wqk_sb[:], wqk_d.ap()[:])
            else:
                nc.scalar.activation(anchor[:], vgl[0:1, 4, 0, 0:16], COPY)
                nc.scalar.dma_start(xq[2][:], xq_d[2].ap()[:])
                nc.scalar.dma_start(xq[3][:], xq_d[3].ap()[:])
                nc.scalar.dma_start(wp_sb[:], wp_d.ap()[:])

        # ---------------- virtual clocks + filler pump ----------------
        clk = {"pe": 0.0, "act": 0.0}

        def pe(ns):
            clk["pe"] += ns

        fill_hi = deque()   # high-priority filler (deadline-bound)
        fill_lo = deque()

        def pump(target):
            while clk["pe"] < target:
                if fill_hi:
                    fill_hi.popleft()()
                elif fill_lo:
                    fill_lo.popleft()()
                else:
                    break

        # ------- phase 1 units (emitted as per-matmul thunks) -------
        def v_unit_thunks(q, tb):
            st = {}

            def first():
                st["pv"] = aux_pool.tile([128, 512], F32, name="pv",
                                         tag="aux")
                nc.tensor.matmul(st["pv"][:],
                                 ones_q[:, tb * 128:(tb + 1) * 128],
                                 bv_sb[:], start=True, stop=False)
                pe(512 * PE_NS + MMF)

            def mid(c):
                nc.tensor.matmul(
                    st["pv"][:], xq[q][:, c, tb * 128:(tb + 1) * 128],
                    wv_sb[:, c, :], start=False, stop=(c == 7))
                pe(512 * PE_NS + MMF)

            def last():
                j = q * 4 + tb
                vg = vgl if j < 8 else vgh
                nc.vector.tensor_copy(
                    vg[:, j % 8, :, 0:64],
                    st["pv"][:].rearrange("p (h d) -> p h d", h=HG))
            return ([first] + [lambda c=c: mid(c) for c in range(8)]
                    + [last])

        def qk_unit_thunks(q, m):
            st = {}

            def mid(c):
                if c == 0:
                    st["pqk"] = aux_pool.tile([128, 512], F32, name="pqk",
                                              tag="aux")
                nc.tensor.matmul(
                    st["pqk"][:], wqk_sb[:, c, m * 128:(m + 1) * 128],
                    xq[q][:, c, :], start=(c == 0), stop=(c == 7))
                pe(512 * PE_NS + MMF)

            def last():
                half = [qTl, qTh, kTl, kTh][(m >= 4) * 2 + (q >= 2)]
                dst = half[m % 4]
                nc.vector.tensor_scalar(
                    out=dst[:, (q % 2) * 512:(q % 2) * 512 + 512],
                    in0=st["pqk"][:],
                    scalar1=bqk_sb[:, m:m + 1], scalar2=None, op0=ADD)
            return [lambda c=c: mid(c) for c in range(8)] + [last]

        def v_unit(q, tb):
            for t in v_unit_thunks(q, tb):
                t()

        def qk_unit(q, m):
            for t in qk_unit_thunks(q, m):
                t()

        # ---------------- attention steps ----------------
        # Unit kinds: 'A' = q<1024 triangle (j 0-7), 'R' = q>=1024 rectangle
        # (k<1024, j 0-7), 'B' = q>=1024 diagonal triangle (j 8-15).
        KIND = {
            "A": dict(c2=0, js=range(0, 8), stop0=3, stop1=7),
            "R": dict(c2=1, js=range(0, 8), stop0=7, stop1=7),
            "B": dict(c2=1, js=range(8, 16), stop0=11, stop1=15),
        }

        def qk_step(h, c2, j, ps_s):
            hp, hh = h // 2, h % 2
            part = slice(64 * hh, 64 * (hh + 1))
            dead = max(0, (j - 8 * c2) * 128)
            diag = j >= 8 * c2
            qTt = (qTl if c2 == 0 else qTh)[hp]
            kTt = (kTl if j < 8 else kTh)[hp]
            kb = kTt[part, (j % 8) * 128:(j % 8) * 128 + 128]
            if dead < 512:
                nc.tensor.matmul(ps_s[:, dead:512], kb,
                                 qTt[part, dead:512],
                                 start=True, stop=not diag)
                pe((512 - dead) * PE_NS + MMF)
                if diag:
                    nc.tensor.matmul(ps_s[:, dead:dead + 128], utri_mask,
                                     id128, start=False, stop=True,
                                     skip_group_check=True)
                    pe(128 * PE_NS + MMF)
                nc.tensor.matmul(ps_s[:, 512:1024], kb,
                                 qTt[part, 512:1024],
                                 start=True, stop=True)
                pe(512 * PE_NS + MMF)
            else:
                lo = dead
                nc.tensor.matmul(ps_s[:, lo:1024], kb,
                                 qTt[part, lo:1024],
                                 start=True, stop=not diag)
                pe((1024 - lo) * PE_NS + MMF)
                if diag:
                    nc.tensor.matmul(ps_s[:, lo:lo + 128], utri_mask, id128,
                                     start=False, stop=True,
                                     skip_group_check=True)
                    pe(128 * PE_NS + MMF)

        def exp_step(c2, j, ps_s):
            dead = max(0, (j - 8 * c2) * 128)
            att_t = att_pool.tile([128, 1024], BF16, tag="att")
            nc.scalar.activation(att_t[:, dead:1024], ps_s[:, dead:1024],
                                 EXP, scale=0.125)
            clk["act"] = (max(clk["act"], clk["pe"] + 150.0)
                          + (1024 - dead) * ACT_NS + ACT_FIX)
            return att_t

        def av_step(h, kind, j, y0, y1, att_t):
            k = KIND[kind]
            c2, j0 = k["c2"], k["js"][0]
            dead = max(0, (j - 8 * c2) * 128)
            va = (vgl if j < 8 else vgh)[:, j % 8, h, :]
            if dead < 512:
                nc.tensor.matmul(y0[:, dead:512], va, att_t[:, dead:512],
                                 start=(j == j0), stop=(j == k["stop0"]))
                pe((512 - dead) * PE_NS + MMF)
            lo = max(512, dead)
            nc.tensor.matmul(y1[:, lo - 512:512], va, att_t[:, lo:1024],
                             start=(j == j0), stop=(j == k["stop1"]))
            pe((1024 - lo) * PE_NS + MMF)

        def inject_rect(h, y0, y1):
            """Add the evacuated rectangle partials into the B-triangle's
            accumulation group via identity matmuls."""
            nc.tensor.matmul(y0[:], id128[0:65, 0:65], yur[h][:, 0:512],
                             start=False, stop=False, skip_group_check=True)
            nc.tensor.matmul(y1[:], id128[0:65, 0:65], yur[h][:, 512:1024],
                             start=False, stop=False, skip_group_check=True)
            pe(1024 * PE_NS + 2 * MMF)

        def evacuate(dst, y0, y1):
            nc.vector.tensor_copy(dst[:, 0:512], y0[:])
            nc.vector.tensor_copy(dst[:, 512:1024], y1[:])

        # ---------------- softmax normalize (SBUF-based filler) ---------
        def norm_unit(h, cch, src):
            """yT[d, q] = src[d, q] / src[64, q] for one 512-col chunk."""
            sl = slice((cch % 2) * 512, (cch % 2) * 512 + 512)
            ps_b = aux_pool.tile([64, 512], F32, name="ps_b", tag="aux")
            nc.tensor.matmul(ps_b[:], ones65[64:65, :], src[64:65, sl],
                             start=True, stop=True)
            pe(512 * PE_NS + MMF)
            inv = nrm_pool.tile([64, 512], F32, tag="inv")
            nc.vector.reciprocal_approx_fast(inv[:], ps_b[:])
            ct, hh = h // 2, h % 2
            yTt = (yTl if cch < 2 else yTh)[ct]
            dsl = slice((cch % 2) * 512, (cch % 2) * 512 + 512)
            if hh == 0:
                nc.vector.tensor_tensor(
                    out=yTt[0:64, dsl], in0=src[0:64, sl], in1=inv[:],
                    op=MUL)
            else:
                ystg = nrm_pool.tile([64, 512], BF16, tag="ystg")
                nc.vector.tensor_tensor(
                    out=ystg[:], in0=src[0:64, sl], in1=inv[:], op=MUL)
                nc.sync.dma_start(yTt[64:128, dsl], ystg[:])

        # ---------------- output projection ----------------
        osb = {}

        def proj_unit_thunks(tb, ch, on_act=False):
            st = {}
            yTt = yTl if tb < 8 else yTh

            def mid(ct):
                if ct == 0:
                    if ch == 0:
                        osb[tb] = out_pool.tile([128, 1024], F32,
                                                name="o_sb", tag="o_sb")
                    st["pp"] = aux_pool.tile([128, 512], F32, name="pp",
                                             tag="aux")
                nc.tensor.matmul(
                    st["pp"][:], yTt[ct][:, (tb % 8) * 128:(tb % 8) * 128 + 128],
                    wp_sb[:, ct, ch * 512:(ch + 1) * 512],
                    start=(ct == 0), stop=(ct == 3))
                pe(512 * PE_NS + MMF)

            def last():
                dst = osb[tb][:, ch * 512:(ch + 1) * 512]
                if on_act:
                    nc.scalar.activation(dst, st["pp"][:], COPY)
                else:
                    nc.vector.tensor_copy(dst, st["pp"][:])
                if ch == 1:
                    nc.sync.dma_start(
                        out_d.ap()[tb * 128:(tb + 1) * 128, :],
                        osb.pop(tb)[:])
            return [lambda ct=ct: mid(ct) for ct in range(4)] + [last]

        def proj_unit(tb, ch, on_act=False):
            for t in proj_unit_thunks(tb, ch, on_act):
                t()

        # ---------------- attention era pipeline ----------------
        def attn_era(units, margin=300.0, on_unit_done=None, pre_unit=None):
            """units: list of (h, kind). Pipelined emission: QK/exp one step
            ahead of AV; filler pumped before each AV."""
            steps = [(h, kind, j)
                     for h, kind in units for j in KIND[kind]["js"]]
            n = len(steps)
            state = {}
            exp_done = {}
            att_of = {}
            for idx in range(n + 2):
                if idx < n:
                    h, kind, j = steps[idx]
                    if j == KIND[kind]["js"][0]:
                        if pre_unit is not None:
                            pre_unit(h, kind)
                        state[(h, kind)] = (
                            ps_y_pool.tile([65, 512], F32, name="ps_y0",
                                           tag="ps_y"),
                            ps_y_pool.tile([65, 512], F32, name="ps_y1",
                                           tag="ps_y"))
                    ps_s = ps_s_pool.tile([128, 1024], F32, name="ps_s",
                                          tag="ps_s")
                    qk_step(h, KIND[kind]["c2"], j, ps_s)
                    att_of[idx] = exp_step(KIND[kind]["c2"], j, ps_s)
                    exp_done[idx] = clk["act"]
                if idx >= 2:
                    ph, pkind, pj = steps[idx - 2]
                    pump(exp_done[idx - 2] + margin)
                    y0, y1 = state[(ph, pkind)]
                    av_step(ph, pkind, pj, y0, y1, att_of.pop(idx - 2))
                    if pkind == "B" and pj == 8:
                        inject_rect(ph, y0, y1)
                    if pj == KIND[pkind]["js"][-1]:
                        dst = {"A": yua, "R": yur, "B": yub}[pkind][ph]
                        evacuate(dst, y0, y1)
                        del state[(ph, pkind)]
                        if on_unit_done is not None:
                            on_unit_done(ph, pkind)

        # ---------------- orchestration ----------------
        # Phase-1 lead: quarters 0-1 straight through.
        for tb in range(4):
            v_unit(0, tb)
        late_dmas(0)
        for tb in range(4):
            v_unit(1, tb)
        late_dmas(1)
        for q in range(2):
            for m in range(8):
                qk_unit(q, m)

        # Era-A filler: qT quarters 2-3 first (needed by the first rectangle
        # at era-A step 8), then v/kT quarters 2-3 (needed by era B), then
        # normalize units as their inputs are evacuated.
        for q in range(2, 4):
            for m in range(4):
                fill_hi.extend(qk_unit_thunks(q, m))
        for q in range(2, 4):
            for tb in range(4):
                fill_lo.extend(v_unit_thunks(q, tb))
            for m in range(4, 8):
                fill_lo.extend(qk_unit_thunks(q, m))

        prev_done = []

        def on_unit_done_a(h, kind):
            # Release the previous unit's normalize work (one-unit delay so
            # the evacuation copies are certainly complete when pumped).
            if prev_done:
                p_h, p_kind = prev_done.pop()
                if p_kind == "A":
                    fill_lo.append(lambda: norm_unit(p_h, 0, yua[p_h]))
                    fill_lo.append(lambda: norm_unit(p_h, 1, yua[p_h]))
            if kind != "R":
                prev_done.append((h, kind))

        def pre_unit_a(h, kind):
            # The first rectangle needs qT quarters 2-3 complete.
            if kind == "R" and h == 0:
                while fill_hi:
                    fill_hi.popleft()()

        units_a = []
        for h in range(HG):
            units_a += [(h, "A"), (h, "R")]
        attn_era(units_a, on_unit_done=on_unit_done_a, pre_unit=pre_unit_a)

        # Boundary: drain remaining quarter/normalize filler (era B needs
        # v_aug j 8-15, kT quarters 2-3 and all A-norms), PE-contiguous.
        while fill_hi:
            fill_hi.popleft()()
        while fill_lo:
            fill_lo.popleft()()
        xq_pool.release()
        wv_pool.release()
        wqk_pool.release()

        # Era B: diagonal triangles. Filler = output projection rows
        # t < 1024 plus B-normalize units as heads complete.
        for tb in range(8):
            for ch in range(2):
                fill_lo.extend(proj_unit_thunks(tb, ch))

        def on_unit_done_b(h, kind):
            if prev_done:
                p_h, _ = prev_done.pop()
                fill_lo.append(lambda: norm_unit(p_h, 2, yub[p_h]))
                fill_lo.append(lambda: norm_unit(p_h, 3, yub[p_h]))
            prev_done.append((h, kind))

        prev_done.clear()
        order_b = [1, 0, 3, 2, 5, 4, 7, 6]
        attn_era([(h, "B") for h in order_b], on_unit_done=on_unit_done_b)
        while fill_lo:
            fill_lo.popleft()()
        while prev_done:
            p_h, _ = prev_done.pop()
            norm_unit(p_h, 2, yub[p_h])
            norm_unit(p_h, 3, yub[p_h])

        # Tail: rows t >= 1024; psum->sbuf copies ride the idle ACT engine.
        for tb in range(8, 16):
            for ch in range(2):
                proj_unit(tb, ch, on_act=True)

    nc.compile()
    return nc


def _get_nc():
    if "nc" not in _NC_CACHE:
        _register_ntff_hook()
        _NC_CACHE["nc"] = _build()
    return _NC_CACHE["nc"]


def kernel(x, w_attn, b_attn, w_proj, b_proj, _run_kwargs=None):
    import ml_dtypes
    from concourse.bass_utils import run_bass_kernel_spmd

    bf16 = ml_dtypes.bfloat16
    x = np.asarray(x, dtype=np.float32)
    w_attn = np.asarray(w_attn, dtype=np.float32)
    b_attn = np.asarray(b_attn, dtype=np.float32)
    w_proj = np.asarray(w_proj, dtype=np.float32)
    b_proj = np.asarray(b_proj, dtype=np.float32)

    cst = np.concatenate(
        [np.triu(np.ones((128, 128), dtype=np.float32), 1) * (-1000.0),
         np.eye(128, dtype=np.float32)], axis=1).astype(bf16)

    nc = _get_nc()
    in_maps = []
    for core in range(NCORES):
        b, g = divmod(core, 2)
        cs = slice(g * CG, (g + 1) * CG)
        xs = np.ascontiguousarray(
            x[b].T.reshape(8, 128, 4, 512).transpose(1, 2, 0, 3)).astype(bf16)
        wqk = np.concatenate(
            [w_attn[:, cs], w_attn[:, C + g * CG: C + (g + 1) * CG]], axis=1)
        bqk = np.concatenate(
            [b_attn[cs], b_attn[C + g * CG: C + (g + 1) * CG]])
        im = {
            "wqk": np.ascontiguousarray(
                wqk.reshape(8, 128, 1024).transpose(1, 0, 2)).astype(bf16),
            "wv": np.ascontiguousarray(
                w_attn[:, 2 * C + g * CG: 2 * C + (g + 1) * CG]
                .reshape(8, 128, 512).transpose(1, 0, 2)).astype(bf16),
            "wp": np.ascontiguousarray(
                w_proj[cs, :].reshape(4, 128, 1024)
                .transpose(1, 0, 2)).astype(bf16),
            "bqk": np.ascontiguousarray(
                bqk.reshape(8, 128).T).astype(np.float32),
            "bv": b_attn[2 * C + g * CG: 2 * C + (g + 1) * CG]
                .reshape(1, 512).astype(bf16),
            "cst": cst,
        }
        for q in range(4):
            im[f"xq{q}"] = np.ascontiguousarray(xs[:, q]).astype(bf16)
        in_maps.append(im)

    res = run_bass_kernel_spmd(nc, in_maps, core_ids=list(range(NCORES)),
                               **(_run_kwargs or {}))
    out = np.empty((B, T, C), dtype=np.float32)
    for b in range(B):
        out[b] = res.results[2 * b]["out"] + res.results[2 * b + 1]["out"] + b_proj
    if _run_kwargs:
        kernel.last_results = res
    return out
